# revision 1
# baseline (speedup 1.0000x reference)
"""Fused BatchNorm1d(train) + block-diagonal GEMM + tanh + residual for TRN2.

  out = tanh(batchnorm(x) @ block_diag(W) + bias) + x,  x: [16384, 4096] fp32

Sharding: expert-style along features. Each of the 8 cores owns 512
features = 4 independent 128x128 blocks, and the full batch, so batch
stats need no collective.

Math: fold normalization into the weights. With s = gamma*rsqrt(var+eps),
t = beta - mean*s:
  y_p = xn_p @ W_p = x_p @ (s_p * W_p) + (t_p @ W_p)
so pass 2 is a plain GEMM with W'_p = s_p*W_p plus a per-output-feature
constant bias'' = bias + t@W, then tanh, then +x.

Pipeline per core (128 row-tiles of [128 batch, 512 feat]):
  Pass 1: DMA x in; cast to bf16 (ACT); one [128,129] matmul per block
          accumulates Gram (sum x^2 on diag) + batch sums in PSUM.
          Optionally PE-transposes some tiles (fp32, exact) and parks
          xT in SBUF for pass 2.
  Finalize: diag/sums -> mean/var -> s, t; scale W on ACT; build bias''
          and split into 3 bf16 rows for a K=3 PSUM bias-broadcast matmul.
  Pass 2: per row-tile: PE-transpose x blocks (fp32) unless parked;
          bias-preload matmul + 4 fp32 GEMMs into one PSUM bank; ACT
          tanh (PSUM->SBUF); DVE residual add; DMA out.
"""

import os
import sys

import numpy as np

for _p in ("/opt/trn_rl_repo", "/root/.axon_site/_ro/trn_rl_repo",
           "/root/.axon_site/_ro/pypackages", "/root/.axon_site"):
    if _p not in sys.path and os.path.isdir(_p):
        sys.path.append(_p)

import ml_dtypes  # noqa: E402
import concourse.tile as tile  # noqa: E402
from concourse import bacc, mybir  # noqa: E402
from concourse.bass_utils import run_bass_kernel_spmd  # noqa: E402

B = 16384          # batch
F = 4096           # features
NPART = 32         # independent blocks
D = 128            # block size
NCORES = 8
FS = F // NCORES   # features per core = 512
NBLK = FS // D     # blocks per core = 4
NT = B // 128      # row-tiles per core = 128
EPS = 1e-5

# Tunables (env-overridable for experiments)
T_RES = int(os.environ.get("KRN_T", "20"))   # xT-resident row-tiles
X_RES = int(os.environ.get("KRN_X", "32"))   # x-resident row-tiles
S2 = int(os.environ.get("KRN_S2", "2"))      # pass-2 super-tile
S1 = int(os.environ.get("KRN_S1", "4"))      # pass-1 super-tile
STATS_FP32 = os.environ.get("KRN_STATS_FP32", "0") == "1"
BUFS = int(os.environ.get("KRN_BUFS", "4"))  # pipeline depth for stream pools
EVAC2_ACT = os.environ.get("KRN_EVAC2_ACT", "0") == "1"
EVAC2_ALT = os.environ.get("KRN_EVAC2_ALT", "1") == "1"
HOIST = int(os.environ.get("KRN_HOIST", "16"))  # P2 supertile loads hoisted over finalize
OUT_ACT_DMA = os.environ.get("KRN_OUT_ACT", "0") == "1"  # out writes on ACT HWDGE ring
P2LEAD = int(os.environ.get("KRN_P2LEAD", "0"))  # T-resident supertiles moved to P2 front

_CACHE: dict = {}


def _residency_maps():
    """Spread X-resident supertiles (S1 granularity) and T-resident tiles
    (tile granularity, among non-X tiles) evenly across the pass."""
    n_sup = NT // S1
    x_sup_cnt = min(X_RES // S1, n_sup)
    x_sups = set()
    acc = 0.0
    for s in range(n_sup):
        acc += x_sup_cnt / n_sup
        if acc >= 1.0 - 1e-9:
            acc -= 1.0
            x_sups.add(s)
    x_tiles = {t for t in range(NT) if (t // S1) in x_sups}
    rest = [t for t in range(NT) if t not in x_tiles]
    t_tiles = set()
    acc = 0.0
    for t in rest:
        acc += min(T_RES, len(rest)) / len(rest)
        if acc >= 1.0 - 1e-9:
            acc -= 1.0
            t_tiles.add(t)
    # bias the tail: force the last TAILT non-X tiles to be T-resident so the
    # drain chain ends with transpose-free tiles (swap out earliest T tiles)
    tailt = int(os.environ.get("KRN_TAILT", "6"))
    tail = [t for t in reversed(range(NT)) if t not in x_tiles][:tailt]
    for t in tail:
        if t not in t_tiles and t_tiles:
            t_tiles.remove(min(t_tiles))
            t_tiles.add(t)
    x_slot = {t: i for i, t in enumerate(sorted(x_tiles))}
    t_slot = {t: i for i, t in enumerate(sorted(t_tiles))}
    return x_tiles, x_slot, t_tiles, t_slot


def _emit_body(nc, tc, ctx, pools, consts, x_d, out_d, it):
    """One full iteration: stats pass + finalize + apply pass, x_d -> out_d."""
    dt = mybir.dt
    (singles, p1_pool, bf_pool, stats_ps, xt_ps, y_ps, xt_work, p2_pool,
     o_pool, fin) = pools
    (ident, ones3, w_orig_f, bias_f, gcol_f, btcol_f) = consts
    x_tiles, x_slot, t_tiles, t_slot = _residency_maps()

    def dram_rows(ap, t0, n):
        return ap[t0 * 128:(t0 + n) * 128, :].rearrange("(a p) f -> p a f", p=128)

    xt_res_t = {t: singles.tile([128, FS], dt.float32, tag=f"xtr{t_slot[t]}",
                                name=f"xtr{t_slot[t]}_{it}") for t in t_tiles}
    x_res_sup = {}
    for t in sorted(x_tiles):
        if t % S1 == 0:
            x_res_sup[t] = singles.tile([128, S1, FS], dt.float32,
                                        tag=f"xr{x_slot[t]}",
                                        name=f"xr{x_slot[t]}_{it}")

    def xt_res_slice(t):
        return xt_res_t[t]

    # ---------------- Pass 1: stats (+ optional transposes) -------------
    sdt = dt.float32 if STATS_FP32 else dt.bfloat16
    gram = [stats_ps.tile([D, D + 1], dt.float32, tag=f"gram{p}",
                          name=f"gram{p}_{it}") for p in range(NBLK)]

    for st in range(NT // S1):
        t0 = st * S1
        if t0 in x_tiles:
            x_src_sup = x_res_sup[t0]
        else:
            x_src_sup = p1_pool.tile([128, S1, FS], dt.float32, tag="x1",
                                     name=f"x1_{it}_{st}")
        nc.sync.dma_start(out=x_src_sup, in_=dram_rows(x_d, t0, S1))

        for k in range(S1):
            t = t0 + k
            x_t = x_src_sup[:, k, :]
            xb = bf_pool.tile([128, NBLK, D + 1], sdt, tag="xb",
                              name=f"xb_{it}_{t}")
            nc.scalar.copy(
                out=xb[:, :, 0:D],
                in_=x_t.rearrange("p (blk d) -> p blk d", blk=NBLK))
            nc.gpsimd.memset(xb[:, :, D:D + 1], 1.0)
            for p in range(NBLK):
                nc.tensor.matmul(
                    gram[p], lhsT=xb[:, p, 0:D], rhs=xb[:, p, :],
                    start=(t == 0), stop=(t == NT - 1))
            if t in t_tiles:
                xt_p = xt_ps.tile([128, FS], dt.float32, tag="xtp",
                                  name=f"xtp1_{it}_{t}")
                for p in range(NBLK):
                    nc.tensor.transpose(
                        xt_p[:, p * D:(p + 1) * D],
                        x_t[:, p * D:(p + 1) * D], ident)
                nc.vector.tensor_copy(out=xt_res_slice(t), in_=xt_p)

    # -------- hoist first pass-2 streamed loads over the finalize barrier
    hoisted = {}
    n_hoist = 0
    st = 0
    while n_hoist < HOIST and st < NT // S2:
        t0 = st * S2
        if t0 not in x_tiles:
            x_sup = p2_pool.tile([128, S2, FS], dt.float32, tag="x2",
                                 name=f"x2h_{it}_{st}")
            nc.sync.dma_start(out=x_sup, in_=dram_rows(x_d, t0, S2))
            hoisted[st] = x_sup
            n_hoist += 1
        st += 1

    # ---------------- Finalize: stats -> scaled weights ------------------
    def ftile(nm, shape=(D, NBLK)):
        return fin.tile(list(shape), dt.float32, tag=nm, name=f"{nm}_{it}")

    sums = ftile("sums")
    ssq = ftile("ssq")
    for p in range(NBLK):
        nc.vector.tensor_copy(out=sums[:, p:p + 1], in_=gram[p][:, D:D + 1])
        dtmp = fin.tile([D, D], dt.float32, tag="dtmp", name=f"dtmp{p}_{it}")
        nc.vector.tensor_mul(dtmp, gram[p][:, 0:D], ident)
        nc.vector.tensor_reduce(
            out=ssq[:, p:p + 1], in_=dtmp, axis=mybir.AxisListType.X,
            op=mybir.AluOpType.add)

    mean = ftile("mean")
    nc.scalar.mul(mean, sums, 1.0 / B)
    var = ftile("var")
    nc.scalar.mul(var, ssq, 1.0 / B)
    m2 = ftile("m2")
    nc.vector.tensor_mul(m2, mean, mean)
    nc.vector.tensor_sub(var, var, m2)
    veps = ftile("veps")
    nc.vector.tensor_scalar_add(veps, var, EPS)
    std = ftile("std")
    nc.scalar.sqrt(std, veps)
    rstd = ftile("rstd")
    nc.vector.reciprocal(rstd, std)
    nt1 = ftile("nt1")
    nc.vector.tensor_mul(nt1, veps, rstd)
    nc.vector.tensor_mul(nt1, nt1, rstd)          # v*r^2
    nc.vector.tensor_scalar(nt1, nt1, -0.5, 1.5,
                            mybir.AluOpType.mult, mybir.AluOpType.add)
    nc.vector.tensor_mul(rstd, rstd, nt1)         # r *= 1.5 - 0.5*v*r^2

    s_c = ftile("s_c")
    nc.vector.tensor_mul(s_c, gcol_f, rstd)
    t_c = ftile("t_c")
    nc.vector.tensor_mul(t_c, mean, s_c)
    nc.vector.tensor_sub(t_c, btcol_f, t_c)       # t = beta - mean*s

    w_s = singles.tile([D, NBLK, D], dt.float32, tag="w_s", name=f"w_s_{it}")
    c_ps = stats_ps.tile([1, FS], dt.float32, tag="gram0", name=f"c_ps_{it}")
    for p in range(NBLK):
        nc.scalar.activation(
            out=w_s[:, p, :], in_=w_orig_f[:, p, :],
            func=mybir.ActivationFunctionType.Copy, scale=s_c[:, p:p + 1])
        nc.tensor.matmul(c_ps[:, p * D:(p + 1) * D], lhsT=t_c[:, p:p + 1],
                         rhs=w_orig_f[:, p, :], start=True, stop=True)
    bias2 = ftile("bias2", (1, FS))
    nc.vector.tensor_copy(out=bias2, in_=c_ps)
    nc.vector.tensor_add(bias2, bias2, bias_f)
    # split bias'' into 3 bf16 components (sum reconstructs ~fp32 exactly)
    bias_hl = singles.tile([3, FS], dt.bfloat16, tag="bias_hl",
                           name=f"bias_hl_{it}")
    rem = ftile("rem", (1, FS))
    rem2 = ftile("rem2", (1, FS))
    bc0 = fin.tile([1, FS], dt.bfloat16, tag="bc0", name=f"bc0_{it}")
    bc1 = fin.tile([1, FS], dt.bfloat16, tag="bc1", name=f"bc1_{it}")
    bc2 = fin.tile([1, FS], dt.bfloat16, tag="bc2", name=f"bc2_{it}")
    nc.vector.tensor_copy(out=bc0, in_=bias2)
    nc.vector.tensor_sub(rem, bias2, bc0)
    nc.vector.tensor_copy(out=bc1, in_=rem)
    nc.vector.tensor_sub(rem2, rem, bc1)
    nc.vector.tensor_copy(out=bc2, in_=rem2)
    for _i, _bc in enumerate([bc0, bc1, bc2]):
        nc.gpsimd.dma_start(out=bias_hl[_i:_i + 1, :], in_=_bc)

    # ---------------- Pass 2: GEMM + tanh + residual ---------------------
    sts = sorted(range(NT // S2),
                 key=lambda s: 0 if (s * S2) in t_tiles else 1)
    order = sts[:P2LEAD] + [s for s in range(NT // S2) if s not in sts[:P2LEAD]]
    for st in order:
        t0 = st * S2
        if st in hoisted:
            x_sup = hoisted[st]
        elif t0 in x_tiles:
            base = (t0 // S1) * S1
            k0 = t0 - base
            x_sup = x_res_sup[base][:, k0:k0 + S2, :]
        else:
            x_sup = p2_pool.tile([128, S2, FS], dt.float32, tag="x2",
                                 name=f"x2_{it}_{st}")
            nc.sync.dma_start(out=x_sup, in_=dram_rows(x_d, t0, S2))
        o_sup = o_pool.tile([128, S2, FS], dt.float32, tag="o2",
                            name=f"o2_{it}_{st}")

        for k in range(S2):
            t = t0 + k
            x_t = x_sup[:, k, :]
            if t in t_tiles:
                xt = xt_res_slice(t)
            else:
                xt_p = xt_ps.tile([128, FS], dt.float32, tag="xtp",
                                  name=f"xtp2_{it}_{t}")
                for p in range(NBLK):
                    nc.tensor.transpose(
                        xt_p[:, p * D:(p + 1) * D],
                        x_t[:, p * D:(p + 1) * D], ident)
                xt = xt_work.tile([128, FS], dt.float32, tag="xtw",
                                  name=f"xtw_{it}_{t}")
                if EVAC2_ACT or (EVAC2_ALT and t % 2 == 0):
                    nc.scalar.copy(out=xt, in_=xt_p)
                else:
                    nc.vector.tensor_copy(out=xt, in_=xt_p)

            y = y_ps.tile([128, FS], dt.float32, tag=f"gram{t % NBLK}",
                          name=f"y_{it}_{t}")
            nc.tensor.matmul(y, lhsT=ones3, rhs=bias_hl, start=True, stop=False)
            for p in range(NBLK):
                nc.tensor.matmul(
                    y[:, p * D:(p + 1) * D], lhsT=xt[:, p * D:(p + 1) * D],
                    rhs=w_s[:, p, :], start=False, stop=(p == NBLK - 1))
            o_t = o_sup[:, k, :]
            nc.scalar.activation(out=o_t, in_=y,
                                 func=mybir.ActivationFunctionType.Tanh)
            nc.vector.tensor_add(o_t, o_t, x_t)

        if OUT_ACT_DMA:
            nc.scalar.dma_start(out=dram_rows(out_d, t0, S2), in_=o_sup)
        else:
            nc.sync.dma_start(out=dram_rows(out_d, t0, S2), in_=o_sup)


def build(chain=1):
    """Build + compile the SPMD program. chain>1 loops the body through
    internal DRAM buffers (for slope timing)."""
    nc = bacc.Bacc("TRN2", target_bir_lowering=False, debug=False)
    dt = mybir.dt
    x_d = nc.dram_tensor("x", [B, FS], dt.float32, kind="ExternalInput").ap()
    w_d = nc.dram_tensor("w", [NBLK, D, D], dt.float32, kind="ExternalInput").ap()
    bias_d = nc.dram_tensor("b", [FS], dt.float32, kind="ExternalInput").ap()
    gamma_d = nc.dram_tensor("g", [FS], dt.float32, kind="ExternalInput").ap()
    beta_d = nc.dram_tensor("bt", [FS], dt.float32, kind="ExternalInput").ap()
    id_d = nc.dram_tensor("ident", [D, D], dt.float32, kind="ExternalInput").ap()
    ones3_d = nc.dram_tensor("ones3", [3, D], dt.bfloat16, kind="ExternalInput").ap()
    out_d = nc.dram_tensor("out", [B, FS], dt.float32, kind="ExternalOutput").ap()
    # unused input whose shape depends on chain: breaks HLO/NEFF cache
    # collisions between chain variants (all real in/outs have fixed shapes)
    nc.dram_tensor("salt", [chain, 1], dt.float32, kind="ExternalInput")
    scratch = [nc.dram_tensor(f"scr{i}", [B, FS], dt.float32).ap()
               for i in range(min(chain - 1, 2))]

    import contextlib
    with tile.TileContext(nc) as tc, contextlib.ExitStack() as ctx:
        singles = ctx.enter_context(tc.tile_pool(name="singles", bufs=1))
        p1_pool = ctx.enter_context(tc.tile_pool(name="p1", bufs=int(os.environ.get("KRN_P1B", "3"))))
        bf_pool = ctx.enter_context(tc.tile_pool(name="bf", bufs=BUFS))
        stats_ps = ctx.enter_context(tc.tile_pool(name="stats_ps", bufs=1, space="PSUM"))
        xt_ps = ctx.enter_context(tc.tile_pool(name="xt_ps", bufs=int(os.environ.get("KRN_XTPS", "4")), space="PSUM"))
        y_ps = stats_ps  # y reuses the 4 stats banks (freed after finalize)
        xt_work = ctx.enter_context(tc.tile_pool(name="xt_work", bufs=BUFS))
        p2_pool = ctx.enter_context(tc.tile_pool(name="p2", bufs=int(os.environ.get("KRN_P2B", "8"))))
        o_pool = ctx.enter_context(tc.tile_pool(name="o", bufs=BUFS))
        fin = ctx.enter_context(tc.tile_pool(name="fin", bufs=1))
        pools = (singles, p1_pool, bf_pool, stats_ps, xt_ps, y_ps, xt_work,
                 p2_pool, o_pool, fin)

        ident = singles.tile([D, D], dt.float32, tag="ident", name="ident")
        nc.sync.dma_start(out=ident, in_=id_d)
        ones3 = singles.tile([3, D], dt.bfloat16, tag="ones3", name="ones3")
        nc.sync.dma_start(out=ones3, in_=ones3_d)
        w_orig = singles.tile([D, NBLK, D], dt.float32, tag="w_orig", name="w_orig")
        nc.sync.dma_start(out=w_orig, in_=w_d.rearrange("blk i j -> i blk j"))
        brow = singles.tile([1, FS], dt.float32, tag="brow", name="brow")
        nc.sync.dma_start(out=brow, in_=bias_d[None, :])
        gcol = singles.tile([D, NBLK], dt.float32, tag="gcol", name="gcol")
        nc.gpsimd.dma_start(out=gcol, in_=gamma_d.rearrange("(p i) -> i p", p=NBLK))
        btcol = singles.tile([D, NBLK], dt.float32, tag="btcol", name="btcol")
        nc.gpsimd.dma_start(out=btcol, in_=beta_d.rearrange("(p i) -> i p", p=NBLK))
        consts = (ident, ones3, w_orig, brow, gcol, btcol)

        for it in range(chain):
            src = x_d if it == 0 else scratch[(it - 1) % 2]
            dst = out_d if it == chain - 1 else scratch[it % 2]
            _emit_body(nc, tc, ctx, pools, consts, src, dst, it)

    nc.compile()
    return nc


def _get_nc():
    key = (T_RES, X_RES, S2, S1, STATS_FP32, BUFS, EVAC2_ACT, HOIST, OUT_ACT_DMA, os.environ.get("KRN_P1B"), P2LEAD, os.environ.get("KRN_XTPS"), EVAC2_ALT, os.environ.get("KRN_P2B"), os.environ.get("KRN_TAILT"), 1)
    if key not in _CACHE:
        _CACHE[key] = build(1)
    return _CACHE[key]


# back-compat alias used by test.py
def _build():
    return _get_nc()


def make_in_maps(x, weights, bias, gamma, beta, chain=1):
    ident = np.eye(D, dtype=np.float32)
    ones3 = np.ones((3, D), dtype=ml_dtypes.bfloat16)
    in_maps = []
    for c in range(NCORES):
        f0 = c * FS
        in_maps.append({
            "x": np.ascontiguousarray(x[:, f0:f0 + FS]),
            "w": np.ascontiguousarray(weights[c * NBLK:(c + 1) * NBLK]),
            "b": np.ascontiguousarray(bias[f0:f0 + FS]),
            "g": np.ascontiguousarray(gamma[f0:f0 + FS]),
            "bt": np.ascontiguousarray(beta[f0:f0 + FS]),
            "ident": ident,
            "ones3": ones3,
            "salt": np.zeros((chain, 1), np.float32),
        })
    return in_maps


def kernel(**inputs) -> np.ndarray:
    x = np.ascontiguousarray(inputs["x"], dtype=np.float32)
    weights = np.ascontiguousarray(inputs["weights"], dtype=np.float32)
    bias = np.ascontiguousarray(inputs["bias"], dtype=np.float32)
    gamma = np.ascontiguousarray(inputs["gamma"], dtype=np.float32)
    beta = np.ascontiguousarray(inputs["beta"], dtype=np.float32)

    nc = _get_nc()
    in_maps = make_in_maps(x, weights, bias, gamma, beta)
    res = run_bass_kernel_spmd(nc, in_maps, list(range(NCORES)))
    out = np.concatenate([res.results[c]["out"] for c in range(NCORES)], axis=1)
    return out.astype(np.float32)


if __name__ == "__main__":
    rng = np.random.default_rng(0)
    ins = {
        "x": rng.standard_normal((B, F), dtype=np.float32),
        "weights": (rng.standard_normal((NPART, D, D), dtype=np.float32)
                    / np.sqrt(D)).astype(np.float32),
        "bias": rng.standard_normal(F, dtype=np.float32) * 0.1,
        "gamma": np.ones(F, dtype=np.float32),
        "beta": np.zeros(F, dtype=np.float32),
    }
    out = kernel(**ins)
    xn = (ins["x"] - ins["x"].mean(0)) / np.sqrt(ins["x"].var(0) + EPS)
    xn = xn * ins["gamma"] + ins["beta"]
    y = np.einsum("bpi,pij->bpj", xn.reshape(B, NPART, D),
                  ins["weights"]).reshape(B, F)
    ref = np.tanh(y + ins["bias"]) + ins["x"]
    err = np.abs(out - ref).max()
    print("abs err:", err, "rel:", err / np.abs(ref).max())



# revision 3
# speedup vs baseline: 1.3680x; 1.3680x over previous
"""Fused BatchNorm1d(train) + block-diagonal GEMM + tanh + residual for TRN2.

  out = tanh(batchnorm(x) @ block_diag(W) + bias) + x,  x: [16384, 4096] fp32

Sharding: expert-style along features. Each of the 8 cores owns 512
features = 4 independent 128x128 blocks and the full batch, so batch
stats need no collective. The core's output is produced TRANSPOSED
([512 feat, 16384 batch] per core); the host unshard step concatenates
and transposes back. This lets pass 2 run entirely in feature-major
layout where the bias is a per-partition ACT operand and the residual
operand is the resident transposed activation itself - no second pass
over x, no bias matmul, no output transpose on device.

Math: fold normalization into the weights. With s = gamma*rsqrt(var+eps),
t = beta - mean*s:
  y = xn @ W = x @ (s*W) + (t @ W)
so pass 2 per block is outT = tanh(W_s^T @ xT + bias2_col) + xT with
W_s = s*W (bf16), bias2 = bias + W^T t, xT the bf16-transposed input.

Pipeline per core (128 row-tiles of [128 batch, 512 feat]):
  Pass 1: DMA x in (fp32); ACT-cast to bf16 (+ones column); 4 gram
          matmuls [128,129] accumulate x^T x and batch sums in PSUM
          over the whole pass. Tiles of "T1" chunks are PE-transposed
          (bf16, 1 cyc/row) and parked feature-major in SBUF; tiles of
          "T2" chunks park the bf16 batch-major copy instead and get
          transposed during pass 2 (balances PE across phases).
  Finalize: gram diag/sums -> mean/var -> s,t; W_s = s*W on ACT (bf16);
          bias2 column via 4 tiny fp32 matmuls W^T t.
  Pass 2: per 512-batch-column chunk: (T2: transpose 16 blocks) 4 bf16
          matmuls into PSUM; ACT tanh+bias (per-partition) to SBUF fp32;
          DVE/Pool residual add of xT; DMA out (fp32, transposed layout).

HBM traffic per core is the minimum possible: x read once (32 MB),
out written once (32 MB), nothing re-read. DMA ~186 us is the floor;
all compute engines fit underneath it per phase.
"""

import os
import sys

import numpy as np

for _p in ("/opt/trn_rl_repo", "/root/.axon_site/_ro/trn_rl_repo",
           "/root/.axon_site/_ro/pypackages", "/root/.axon_site"):
    if _p not in sys.path and os.path.isdir(_p):
        sys.path.append(_p)

import ml_dtypes  # noqa: E402
import concourse.tile as tile  # noqa: E402
from concourse import bacc, mybir  # noqa: E402
from concourse.bass_utils import run_bass_kernel_spmd  # noqa: E402

B = 16384          # batch
F = 4096           # features
NPART = 32         # independent blocks
D = 128            # block size
NCORES = 8
FS = F // NCORES   # features per core = 512
NBLK = FS // D     # blocks per core = 4
NT = B // 128      # row-tiles per core = 128
CH = 512           # pass-2 batch-column chunk
NCH = B // CH      # chunks = 32
TPC = CH // 128    # row-tiles per chunk = 4
EPS = 1e-5

# Tunables (env-overridable for experiments)
T2K = int(os.environ.get("KRN_T2K", "16"))   # chunks transposed in pass 2
RS = int(os.environ.get("KRN_RS", "2"))      # residual blocks on DVE (rest Pool)
S1 = int(os.environ.get("KRN_S1", "4"))      # pass-1 supertile (row-tiles)
P1B = int(os.environ.get("KRN_P1B", "3"))    # pass-1 x stream bufs
XBB = int(os.environ.get("KRN_XBB", "4"))    # xb stream bufs
XTPS = int(os.environ.get("KRN_XTPS", "4"))  # transpose PSUM bufs
OB = int(os.environ.get("KRN_OB", "3"))      # output stream bufs
XSB = int(os.environ.get("KRN_XSB", "2"))    # T2 xt rotating bufs

_CACHE: dict = {}


def _t2_chunks():
    """Spread T2K pass-2-transposed chunks evenly over the NCH chunks."""
    t2 = set()
    acc = 0.5
    for c in range(NCH):
        acc += T2K / NCH
        if acc >= 1.0 - 1e-9:
            acc -= 1.0
            t2.add(c)
    return t2


def build():
    nc = bacc.Bacc("TRN2", target_bir_lowering=False, debug=False)
    dt = mybir.dt
    x_d = nc.dram_tensor("x", [B, FS], dt.float32, kind="ExternalInput").ap()
    w_d = nc.dram_tensor("w", [NBLK, D, D], dt.float32, kind="ExternalInput").ap()
    bias_d = nc.dram_tensor("b", [FS], dt.float32, kind="ExternalInput").ap()
    gamma_d = nc.dram_tensor("g", [FS], dt.float32, kind="ExternalInput").ap()
    beta_d = nc.dram_tensor("bt", [FS], dt.float32, kind="ExternalInput").ap()
    id_d = nc.dram_tensor("ident", [D, D], dt.float32, kind="ExternalInput").ap()
    out_d = nc.dram_tensor("out", [FS, B], dt.float32, kind="ExternalOutput").ap()

    t2c = _t2_chunks()
    t1c = [c for c in range(NCH) if c not in t2c]
    t1slot = {c: i for i, c in enumerate(t1c)}
    t2tiles = {4 * c + k for c in t2c for k in range(TPC)}
    t2tslot = {t: i for i, t in enumerate(sorted(t2tiles))}
    n_t1 = len(t1c)

    import contextlib
    with tile.TileContext(nc) as tc, contextlib.ExitStack() as ctx:
        sing = ctx.enter_context(tc.tile_pool(name="sing", bufs=1))
        p1x = ctx.enter_context(tc.tile_pool(name="p1x", bufs=P1B))
        xbp = ctx.enter_context(tc.tile_pool(name="xbp", bufs=XBB))
        ps = ctx.enter_context(tc.tile_pool(name="ps", bufs=1, space="PSUM"))
        xtps = ctx.enter_context(tc.tile_pool(name="xtps", bufs=XTPS, space="PSUM"))
        xsbp = ctx.enter_context(tc.tile_pool(name="xsb", bufs=XSB))
        op = ctx.enter_context(tc.tile_pool(name="op", bufs=OB))
        fin = ctx.enter_context(tc.tile_pool(name="fin", bufs=1))

        # ---------------- constants -------------------------------------
        identf = sing.tile([D, D], dt.float32, tag="identf", name="identf")
        nc.sync.dma_start(out=identf, in_=id_d)
        identb = sing.tile([D, D], dt.bfloat16, tag="identb", name="identb")
        nc.scalar.copy(out=identb, in_=identf)
        w_orig = sing.tile([D, NBLK, D], dt.float32, tag="w_orig", name="w_orig")
        nc.sync.dma_start(out=w_orig, in_=w_d.rearrange("blk i j -> i blk j"))
        gcol = sing.tile([D, NBLK], dt.float32, tag="gcol", name="gcol")
        nc.gpsimd.dma_start(out=gcol, in_=gamma_d.rearrange("(p i) -> i p", p=NBLK))
        btcol = sing.tile([D, NBLK], dt.float32, tag="btcol", name="btcol")
        nc.gpsimd.dma_start(out=btcol, in_=beta_d.rearrange("(p i) -> i p", p=NBLK))
        bcol = sing.tile([D, NBLK], dt.float32, tag="bcol", name="bcol")
        nc.gpsimd.dma_start(out=bcol, in_=bias_d.rearrange("(p i) -> i p", p=NBLK))

        # resident transposed activations for T1 chunks [feat, batch-cols]
        xt_t1 = None
        if n_t1:
            xt_t1 = sing.tile([D, NBLK, n_t1 * CH], dt.bfloat16, tag="xt1",
                              name="xt_t1")
        # resident bf16 batch-major copies for T2 tiles
        xbr = {t: sing.tile([D, NBLK, D + 1], dt.bfloat16, tag=f"xbr{t2tslot[t]}",
                            name=f"xbr{t2tslot[t]}")
               for t in sorted(t2tiles)}

        gram = [ps.tile([D, D + 1], dt.float32, tag=f"g{p}", name=f"gram{p}")
                for p in range(NBLK)]

        # ---------------- pass 1: stats + transposes ---------------------
        for st in range(NT // S1):
            t0 = st * S1
            x_sup = p1x.tile([D, S1, FS], dt.float32, tag="x1", name=f"x1_{st}")
            nc.sync.dma_start(
                out=x_sup,
                in_=x_d[t0 * 128:(t0 + S1) * 128, :].rearrange(
                    "(a p) f -> p a f", p=128))
            for k in range(S1):
                t = t0 + k
                c = t // TPC
                if t in t2tiles:
                    xb = xbr[t]
                else:
                    xb = xbp.tile([D, NBLK, D + 1], dt.bfloat16, tag="xb",
                                  name=f"xb_{t}")
                nc.scalar.copy(
                    out=xb[:, :, 0:D],
                    in_=x_sup[:, k, :].rearrange("p (blk d) -> p blk d",
                                                 blk=NBLK))
                nc.gpsimd.memset(xb[:, :, D:D + 1], 1.0)
                for p in range(NBLK):
                    nc.tensor.matmul(
                        gram[p], lhsT=xb[:, p, 0:D], rhs=xb[:, p, :],
                        start=(t == 0), stop=(t == NT - 1))
                if t not in t2tiles:
                    xtp = xtps.tile([D, NBLK, D], dt.bfloat16, tag="xtp",
                                    name=f"xtp_{t}")
                    for p in range(NBLK):
                        nc.tensor.transpose(xtp[:, p, :], xb[:, p, 0:D], identb)
                    col = t1slot[c] * CH + (t % TPC) * D
                    nc.vector.tensor_copy(out=xt_t1[:, :, col:col + D], in_=xtp)

        # ---------------- finalize: stats -> scaled weights ---------------
        def ftile(nm, shape=(D, NBLK)):
            return fin.tile(list(shape), dt.float32, tag=nm, name=nm)

        sums = ftile("sums")
        ssq = ftile("ssq")
        dtmp = ftile("dtmp", (D, D))
        for p in range(NBLK):
            nc.vector.tensor_copy(out=sums[:, p:p + 1], in_=gram[p][:, D:D + 1])
            nc.vector.tensor_mul(dtmp, gram[p][:, 0:D], identf)
            nc.vector.tensor_reduce(
                out=ssq[:, p:p + 1], in_=dtmp, axis=mybir.AxisListType.X,
                op=mybir.AluOpType.add)

        mean = ftile("mean")
        nc.vector.tensor_scalar_mul(mean, sums, 1.0 / B)
        var = ftile("var")
        nc.vector.tensor_scalar_mul(var, ssq, 1.0 / B)
        m2 = ftile("m2")
        nc.vector.tensor_mul(m2, mean, mean)
        nc.vector.tensor_sub(var, var, m2)
        veps = ftile("veps")
        nc.vector.tensor_scalar_add(veps, var, EPS)
        std = ftile("std")
        nc.scalar.sqrt(std, veps)
        rstd = ftile("rstd")
        nc.vector.reciprocal(rstd, std)
        nt1 = ftile("nt1")
        nc.vector.tensor_mul(nt1, veps, rstd)
        nc.vector.tensor_mul(nt1, nt1, rstd)          # v*r^2
        nc.vector.tensor_scalar(nt1, nt1, -0.5, 1.5,
                                mybir.AluOpType.mult, mybir.AluOpType.add)
        nc.vector.tensor_mul(rstd, rstd, nt1)         # r *= 1.5 - 0.5*v*r^2

        s_c = ftile("s_c")
        nc.vector.tensor_mul(s_c, gcol, rstd)
        t_c = ftile("t_c")
        nc.vector.tensor_mul(t_c, mean, s_c)
        nc.vector.tensor_sub(t_c, btcol, t_c)         # t = beta - mean*s

        w_s = sing.tile([D, NBLK, D], dt.bfloat16, tag="w_s", name="w_s")
        # reuse the (now dead) gram g0 bank for the tiny bias matmul output
        bps = ps.tile([D, NBLK], dt.float32, tag="g0", name="bps")
        for p in range(NBLK):
            nc.scalar.activation(
                out=w_s[:, p, :], in_=w_orig[:, p, :],
                func=mybir.ActivationFunctionType.Copy, scale=s_c[:, p:p + 1])
            nc.tensor.matmul(bps[:, p:p + 1], lhsT=w_orig[:, p, :],
                             rhs=t_c[:, p:p + 1], start=True, stop=True)
        bcol2 = ftile("bcol2")
        nc.vector.tensor_add(bcol2, bps, bcol)        # bias + W^T t

        # ---------------- pass 2: GEMM + tanh + residual ------------------
        outv = out_d.rearrange("(blk p) b -> p blk b", p=D)
        for c in range(NCH):
            if c in t2c:
                xts = xsbp.tile([D, NBLK, CH], dt.bfloat16, tag="xts",
                                name=f"xts_{c}")
                for ti in range(TPC):
                    t = TPC * c + ti
                    xtp = xtps.tile([D, NBLK, D], dt.bfloat16, tag="xtp",
                                    name=f"xtp2_{t}")
                    for p in range(NBLK):
                        nc.tensor.transpose(xtp[:, p, :], xbr[t][:, p, 0:D],
                                            identb)
                    nc.vector.tensor_copy(out=xts[:, :, ti * D:(ti + 1) * D],
                                          in_=xtp)
                xt_view = xts
            else:
                s0 = t1slot[c] * CH
                xt_view = xt_t1[:, :, s0:s0 + CH]

            ys = []
            for p in range(NBLK):
                y = ps.tile([D, CH], dt.float32, tag=f"g{p}", name=f"y_{c}_{p}")
                nc.tensor.matmul(y, lhsT=w_s[:, p, :], rhs=xt_view[:, p, :],
                                 start=True, stop=True)
                ys.append(y)
            o = op.tile([D, NBLK, CH], dt.float32, tag="o", name=f"o_{c}")
            for p in range(NBLK):
                nc.scalar.activation(out=o[:, p, :], in_=ys[p],
                                     func=mybir.ActivationFunctionType.Tanh,
                                     bias=bcol2[:, p:p + 1])
            if RS > 0:
                nc.vector.tensor_add(o[:, 0:RS, :], o[:, 0:RS, :],
                                     xt_view[:, 0:RS, :])
            if RS < NBLK:
                nc.gpsimd.tensor_add(o[:, RS:NBLK, :], o[:, RS:NBLK, :],
                                     xt_view[:, RS:NBLK, :])
            nc.sync.dma_start(out=outv[:, :, c * CH:(c + 1) * CH], in_=o)

    nc.compile()
    return nc


def _get_nc():
    key = (T2K, RS, S1, P1B, XBB, XTPS, OB, XSB)
    if key not in _CACHE:
        _CACHE[key] = build()
    return _CACHE[key]


# back-compat alias used by test.py
def _build():
    return _get_nc()


def make_in_maps(x, weights, bias, gamma, beta):
    ident = np.eye(D, dtype=np.float32)
    in_maps = []
    for c in range(NCORES):
        f0 = c * FS
        in_maps.append({
            "x": np.ascontiguousarray(x[:, f0:f0 + FS]),
            "w": np.ascontiguousarray(weights[c * NBLK:(c + 1) * NBLK]),
            "b": np.ascontiguousarray(bias[f0:f0 + FS]),
            "g": np.ascontiguousarray(gamma[f0:f0 + FS]),
            "bt": np.ascontiguousarray(beta[f0:f0 + FS]),
            "ident": ident,
        })
    return in_maps


def kernel(**inputs) -> np.ndarray:
    x = np.ascontiguousarray(inputs["x"], dtype=np.float32)
    weights = np.ascontiguousarray(inputs["weights"], dtype=np.float32)
    bias = np.ascontiguousarray(inputs["bias"], dtype=np.float32)
    gamma = np.ascontiguousarray(inputs["gamma"], dtype=np.float32)
    beta = np.ascontiguousarray(inputs["beta"], dtype=np.float32)

    nc = _get_nc()
    in_maps = make_in_maps(x, weights, bias, gamma, beta)
    res = run_bass_kernel_spmd(nc, in_maps, list(range(NCORES)))
    # per-core outputs are [FS, B]; unshard = concat along features + transpose
    full_t = np.concatenate(
        [np.asarray(res.results[c]["out"]) for c in range(NCORES)], axis=0)
    return np.ascontiguousarray(full_t.T, dtype=np.float32)


if __name__ == "__main__":
    rng = np.random.default_rng(0)
    ins = {
        "x": rng.standard_normal((B, F), dtype=np.float32),
        "weights": (rng.standard_normal((NPART, D, D), dtype=np.float32)
                    / np.sqrt(D)).astype(np.float32),
        "bias": rng.standard_normal(F, dtype=np.float32) * 0.1,
        "gamma": np.ones(F, dtype=np.float32),
        "beta": np.zeros(F, dtype=np.float32),
    }
    out = kernel(**ins)
    xn = (ins["x"] - ins["x"].mean(0)) / np.sqrt(ins["x"].var(0) + EPS)
    xn = xn * ins["gamma"] + ins["beta"]
    y = np.einsum("bpi,pij->bpj", xn.reshape(B, NPART, D),
                  ins["weights"]).reshape(B, F)
    ref = np.tanh(y + ins["bias"]) + ins["x"]
    err = np.abs(out - ref).max()
    print("abs err:", err, "rel:", err / np.abs(ref).max())


# revision 6
# speedup vs baseline: 1.4505x; 1.0603x over previous
"""Fused BatchNorm1d(train) + block-diagonal GEMM + tanh + residual for TRN2.

  out = tanh(batchnorm(x) @ block_diag(W) + bias) + x,  x: [16384, 4096] fp32

Sharding: expert-style along features. Each of the 8 cores owns 512
features = 4 independent 128x128 blocks and the full batch, so batch
stats need no collective. The core's output is produced TRANSPOSED
([512 feat, 16384 batch] per core); the host unshard step concatenates
and transposes back. Pass 2 then runs in feature-major layout where the
bias is a per-partition ACT operand and the residual operand is the
resident transposed activation - no second pass over x, no bias matmul,
no output transpose on device.

Math: fold normalization into the weights. With s = gamma*rsqrt(var+eps),
t = beta - mean*s:
  outT_p = tanh(W_sp^T @ xT_p + bias2_p) + xT_p,   W_s = s*W (bf16),
  bias2 = bias + W^T t,   xT the bf16-transposed input.

Pipeline per core (128 row-tiles of [128 batch, 512 feat]):
  Pass 1: DMA x in (fp32); cast to bf16 +ones column (ACT/DVE alternating);
          4 gram matmuls [128,129] accumulate x^T x + batch sums in PSUM
          across the pass; 4 bf16 PE transposes park xT in SBUF (16 MB).
  Finalize: gram diag/sums -> mean/var -> s,t; W_s = s*W on ACT (bf16);
          bias2 column via 4 tiny fp32 matmuls W^T t.
  Pass 2: per batch-column chunk: 4 bf16 matmuls into PSUM; ACT tanh+bias
          (per-partition) to SBUF fp32; DVE/Pool residual add of xT; DMA
          out (fp32, transposed layout). Edge chunks are narrow to cut
          the finalize bubble and the drain tail.

HBM traffic per core is the minimum possible: x read once (32 MB), out
written once (32 MB), nothing re-read. DMA ~187 us is the floor; all
compute engines fit underneath it in both phases.
"""

import os
import sys

import numpy as np

for _p in ("/opt/trn_rl_repo", "/root/.axon_site/_ro/trn_rl_repo",
           "/root/.axon_site/_ro/pypackages", "/root/.axon_site"):
    if _p not in sys.path and os.path.isdir(_p):
        sys.path.append(_p)

import ml_dtypes  # noqa: E402
import concourse.tile as tile  # noqa: E402
from concourse import bacc, mybir  # noqa: E402
from concourse.bass_utils import run_bass_kernel_spmd  # noqa: E402

B = 16384          # batch
F = 4096           # features
NPART = 32         # independent blocks
D = 128            # block size
NCORES = 8
FS = F // NCORES   # features per core = 512
NBLK = FS // D     # blocks per core = 4
NT = B // 128      # row-tiles per core = 128
EPS = 1e-5

# Tunables (env-overridable for experiments)
RS = int(os.environ.get("KRN_RS", "3"))      # residual blocks on DVE (rest Pool)
S1 = int(os.environ.get("KRN_S1", "4"))      # pass-1 supertile (row-tiles)
P1B = int(os.environ.get("KRN_P1B", "3"))    # pass-1 x stream bufs
XBB = int(os.environ.get("KRN_XBB", "4"))    # xb stream bufs
XTPS = int(os.environ.get("KRN_XTPS", "2"))  # transpose PSUM bufs
YEX = int(os.environ.get("KRN_YEX", "2"))    # extra PSUM y slots beyond 4
OB = int(os.environ.get("KRN_OB", "3"))      # output stream bufs
CDM = int(os.environ.get("KRN_CDM", "2"))    # every CDM-th tile cast on DVE
EDGE = os.environ.get("KRN_EDGE", "1") == "1"  # narrow first/last p2 chunks

_CACHE: dict = {}


def _chunks():
    """Pass-2 batch-column chunks as (col0, width)."""
    if not EDGE:
        return [(c * 512, 512) for c in range(B // 512)]
    widths = [128, 384] + [512] * ((B - 2 * 512) // 512) + [384, 128]
    out, col = [], 0
    for w in widths:
        out.append((col, w))
        col += w
    assert col == B
    return out


def build():
    nc = bacc.Bacc("TRN2", target_bir_lowering=False, debug=False)
    dt = mybir.dt
    x_d = nc.dram_tensor("x", [B, FS], dt.float32, kind="ExternalInput").ap()
    w_d = nc.dram_tensor("w", [NBLK, D, D], dt.float32, kind="ExternalInput").ap()
    bias_d = nc.dram_tensor("b", [FS], dt.float32, kind="ExternalInput").ap()
    gamma_d = nc.dram_tensor("g", [FS], dt.float32, kind="ExternalInput").ap()
    beta_d = nc.dram_tensor("bt", [FS], dt.float32, kind="ExternalInput").ap()
    id_d = nc.dram_tensor("ident", [D, D], dt.float32, kind="ExternalInput").ap()
    out_d = nc.dram_tensor("out", [FS, B], dt.float32, kind="ExternalOutput").ap()

    import contextlib
    with tile.TileContext(nc) as tc, contextlib.ExitStack() as ctx:
        sing = ctx.enter_context(tc.tile_pool(name="sing", bufs=1))
        p1x = ctx.enter_context(tc.tile_pool(name="p1x", bufs=P1B))
        xbp = ctx.enter_context(tc.tile_pool(name="xbp", bufs=XBB))
        ps = ctx.enter_context(tc.tile_pool(name="ps", bufs=1, space="PSUM"))
        xtps = ctx.enter_context(tc.tile_pool(name="xtps", bufs=XTPS, space="PSUM"))
        op = ctx.enter_context(tc.tile_pool(name="op", bufs=OB))
        fin = ctx.enter_context(tc.tile_pool(name="fin", bufs=1))

        # ---- first x loads before the small consts, to start DMA on x ----
        def load_sup(st):
            t0 = st * S1
            x_sup = p1x.tile([D, S1, FS], dt.float32, tag="x1", name=f"x1_{st}")
            nc.sync.dma_start(
                out=x_sup,
                in_=x_d[t0 * 128:(t0 + S1) * 128, :].rearrange(
                    "(a p) f -> p a f", p=128))
            return x_sup

        sup0 = load_sup(0)

        identf = sing.tile([D, D], dt.float32, tag="identf", name="identf")
        nc.sync.dma_start(out=identf, in_=id_d)
        identb = sing.tile([D, D], dt.bfloat16, tag="identb", name="identb")
        nc.vector.tensor_copy(out=identb, in_=identf)
        w_orig = sing.tile([D, NBLK, D], dt.float32, tag="w_orig", name="w_orig")
        nc.sync.dma_start(out=w_orig, in_=w_d.rearrange("blk i j -> i blk j"))
        gcol = sing.tile([D, NBLK], dt.float32, tag="gcol", name="gcol")
        nc.gpsimd.dma_start(out=gcol, in_=gamma_d.rearrange("(p i) -> i p", p=NBLK))
        btcol = sing.tile([D, NBLK], dt.float32, tag="btcol", name="btcol")
        nc.gpsimd.dma_start(out=btcol, in_=beta_d.rearrange("(p i) -> i p", p=NBLK))
        bcol = sing.tile([D, NBLK], dt.float32, tag="bcol", name="bcol")
        nc.gpsimd.dma_start(out=bcol, in_=bias_d.rearrange("(p i) -> i p", p=NBLK))

        # resident transposed activations [feat, batch] bf16, 16 MB
        xt = sing.tile([D, NBLK, B], dt.bfloat16, tag="xt", name="xt")

        gram = [ps.tile([D, D + 1], dt.float32, tag=f"g{p}", name=f"gram{p}")
                for p in range(NBLK)]

        # ---------------- pass 1: stats + transposes ---------------------
        for st in range(NT // S1):
            x_sup = sup0 if st == 0 else load_sup(st)
            for k in range(S1):
                t = st * S1 + k
                xb = xbp.tile([D, NBLK, D + 1], dt.bfloat16, tag="xb",
                              name=f"xb_{t}")
                xsrc = x_sup[:, k, :].rearrange("p (blk d) -> p blk d", blk=NBLK)
                if CDM > 0 and t % CDM == CDM - 1:
                    nc.vector.tensor_copy(out=xb[:, :, 0:D], in_=xsrc)
                else:
                    nc.scalar.copy(out=xb[:, :, 0:D], in_=xsrc)
                nc.gpsimd.memset(xb[:, :, D:D + 1], 1.0)
                for p in range(NBLK):
                    nc.tensor.matmul(
                        gram[p], lhsT=xb[:, p, 0:D], rhs=xb[:, p, :],
                        start=(t == 0), stop=(t == NT - 1))
                xtp = xtps.tile([D, NBLK, D], dt.bfloat16, tag="xtp",
                                name=f"xtp_{t}")
                for p in range(NBLK):
                    nc.tensor.transpose(xtp[:, p, :], xb[:, p, 0:D], identb)
                col = t * D
                nc.vector.tensor_copy(out=xt[:, :, col:col + D], in_=xtp)

        # ---------------- finalize: stats -> scaled weights ---------------
        def ftile(nm, shape=(D, NBLK)):
            return fin.tile(list(shape), dt.float32, tag=nm, name=nm)

        sums = ftile("sums")
        ssq = ftile("ssq")
        dtmp = ftile("dtmp", (D, D))
        for p in range(NBLK):
            nc.vector.tensor_copy(out=sums[:, p:p + 1], in_=gram[p][:, D:D + 1])
            nc.vector.tensor_mul(dtmp, gram[p][:, 0:D], identf)
            nc.vector.tensor_reduce(out=ssq[:, p:p + 1], in_=dtmp,
                                    axis=mybir.AxisListType.X,
                                    op=mybir.AluOpType.add)

        mean = ftile("mean")
        nc.vector.tensor_scalar_mul(mean, sums, 1.0 / B)
        ex2 = ftile("ex2")
        nc.vector.tensor_scalar_mul(ex2, ssq, 1.0 / B)
        m2 = ftile("m2")
        nc.vector.tensor_mul(m2, mean, mean)
        veps = ftile("veps")
        nc.vector.tensor_sub(veps, ex2, m2)
        nc.vector.tensor_scalar_add(veps, veps, EPS)
        std = ftile("std")
        nc.scalar.sqrt(std, veps)
        rstd = ftile("rstd")
        nc.vector.reciprocal(rstd, std)
        s_c = ftile("s_c")
        nc.vector.tensor_mul(s_c, gcol, rstd)
        t_c = ftile("t_c")
        nc.vector.tensor_mul(t_c, mean, s_c)
        nc.vector.tensor_sub(t_c, btcol, t_c)         # t = beta - mean*s

        w_s = sing.tile([D, NBLK, D], dt.bfloat16, tag="w_s", name="w_s")
        bps = ps.tile([D, NBLK], dt.float32, tag="g0", name="bps")
        for p in range(NBLK):
            nc.scalar.activation(
                out=w_s[:, p, :], in_=w_orig[:, p, :],
                func=mybir.ActivationFunctionType.Copy, scale=s_c[:, p:p + 1])
            nc.tensor.matmul(bps[:, p:p + 1], lhsT=w_orig[:, p, :],
                             rhs=t_c[:, p:p + 1], start=True, stop=True)
        bcol2 = ftile("bcol2")
        nc.vector.tensor_add(bcol2, bps, bcol)        # bias + W^T t

        # ---------------- pass 2: GEMM + tanh + residual ------------------
        outv = out_d.rearrange("(blk p) b -> p blk b", p=D)
        nyt = NBLK + YEX
        yc = 0
        for c0, cw in _chunks():
            ys = []
            for p in range(NBLK):
                y = ps.tile([D, 512], dt.float32, tag=f"g{yc % nyt}",
                            name=f"y_{c0}_{p}")
                yc += 1
                nc.tensor.matmul(y[:, 0:cw], lhsT=w_s[:, p, :],
                                 rhs=xt[:, p, c0:c0 + cw],
                                 start=True, stop=True)
                ys.append(y)
            o = op.tile([D, NBLK, 512], dt.float32, tag="o", name=f"o_{c0}")
            for p in range(NBLK):
                nc.scalar.activation(out=o[:, p, 0:cw], in_=ys[p][:, 0:cw],
                                     func=mybir.ActivationFunctionType.Tanh,
                                     bias=bcol2[:, p:p + 1])
            rs = RS if cw >= 512 else NBLK
            if rs > 0:
                nc.vector.tensor_add(o[:, 0:rs, 0:cw], o[:, 0:rs, 0:cw],
                                     xt[:, 0:rs, c0:c0 + cw])
            if rs < NBLK:
                nc.gpsimd.tensor_add(o[:, rs:NBLK, 0:cw], o[:, rs:NBLK, 0:cw],
                                     xt[:, rs:NBLK, c0:c0 + cw])
            nc.sync.dma_start(out=outv[:, :, c0:c0 + cw], in_=o[:, :, 0:cw])

    nc.compile()
    return nc


def _get_nc():
    key = (RS, S1, P1B, XBB, XTPS, YEX, OB, CDM, EDGE)
    if key not in _CACHE:
        _CACHE[key] = build()
    return _CACHE[key]


# back-compat alias used by test.py
def _build():
    return _get_nc()


def make_in_maps(x, weights, bias, gamma, beta):
    ident = np.eye(D, dtype=np.float32)
    in_maps = []
    for c in range(NCORES):
        f0 = c * FS
        in_maps.append({
            "x": np.ascontiguousarray(x[:, f0:f0 + FS]),
            "w": np.ascontiguousarray(weights[c * NBLK:(c + 1) * NBLK]),
            "b": np.ascontiguousarray(bias[f0:f0 + FS]),
            "g": np.ascontiguousarray(gamma[f0:f0 + FS]),
            "bt": np.ascontiguousarray(beta[f0:f0 + FS]),
            "ident": ident,
        })
    return in_maps


def kernel(**inputs) -> np.ndarray:
    x = np.ascontiguousarray(inputs["x"], dtype=np.float32)
    weights = np.ascontiguousarray(inputs["weights"], dtype=np.float32)
    bias = np.ascontiguousarray(inputs["bias"], dtype=np.float32)
    gamma = np.ascontiguousarray(inputs["gamma"], dtype=np.float32)
    beta = np.ascontiguousarray(inputs["beta"], dtype=np.float32)

    nc = _get_nc()
    in_maps = make_in_maps(x, weights, bias, gamma, beta)
    res = run_bass_kernel_spmd(nc, in_maps, list(range(NCORES)))
    # per-core outputs are [FS, B]; unshard = concat along features + transpose
    full_t = np.concatenate(
        [np.asarray(res.results[c]["out"]) for c in range(NCORES)], axis=0)
    return np.ascontiguousarray(full_t.T, dtype=np.float32)


if __name__ == "__main__":
    rng = np.random.default_rng(0)
    ins = {
        "x": rng.standard_normal((B, F), dtype=np.float32),
        "weights": (rng.standard_normal((NPART, D, D), dtype=np.float32)
                    / np.sqrt(D)).astype(np.float32),
        "bias": rng.standard_normal(F, dtype=np.float32) * 0.1,
        "gamma": np.ones(F, dtype=np.float32),
        "beta": np.zeros(F, dtype=np.float32),
    }
    out = kernel(**ins)
    xn = (ins["x"] - ins["x"].mean(0)) / np.sqrt(ins["x"].var(0) + EPS)
    xn = xn * ins["gamma"] + ins["beta"]
    y = np.einsum("bpi,pij->bpj", xn.reshape(B, NPART, D),
                  ins["weights"]).reshape(B, F)
    ref = np.tanh(y + ins["bias"]) + ins["x"]
    err = np.abs(out - ref).max()
    print("abs err:", err, "rel:", err / np.abs(ref).max())
